# revision 23
# baseline (speedup 1.0000x reference)
"""Causal int8-quantized softmax kernel for Trainium2 (8 NeuronCores).

Problem: x_q [1,16,2048,2048] int32 (int8-valued scores), scale_x/scale_out
[16,2048] f32 per-(head,row) scales.  Computes
    out_q = clip(round(softmax(tril_mask(x_q * sx)) / so), -128, 127) int8
Sharding: 2 heads per core (16 heads / 8 cores); softmax is row-local so no
cross-core communication.

Per-core kernel structure (16 row-tiles of 128 rows, both heads fused per
tile):
  - causal structure: row-tile r only needs columns [0, 128*(r+1)) — the rest
    of the row is exactly 0 in the output and is never loaded, computed, or
    stored (the runtime pre-zeroes output buffers).
  - both heads' [128, W] blocks move in ONE load / ONE store DMA ([p, h, w]
    layout) — halves DMA count, doubles transfer size.
  - diag 128x128 blocks get an additive -2^20 mask (int32) so exp underflows
    to exactly 0 for masked entries.
  - one ACT pass per (head, tile): exp(sx*x) with accum_out giving the row
    sum for free.
  - DVE: factor = 1/(sum*so); out int8 = exp * factor (the f32->int8 convert
    on write rounds-to-nearest and saturates, matching round+clip).
"""

import sys

if "/opt/trn_rl_repo" not in sys.path:
    sys.path.insert(0, "/opt/trn_rl_repo")

import numpy as np

N_CORES = 8
H = 16
H_PER_CORE = H // N_CORES  # 2
S = 2048
P = 128
NT = S // P  # 16 row-tiles per head

# Tile processing order and load-prefetch depth (best of a TimelineSim
# sweep: DMA stays ~100% busy from first load to last store, and the kernel
# tail drains on the smallest tiles).
_ORDER = [1, 5, 9, 13, 15, 11, 7, 3, 2, 6, 10, 14, 12, 8, 4, 0]
_PREFETCH = 4


def _build():
    import concourse.bacc as bacc
    import concourse.mybir as mybir
    import concourse.tile as tile

    nc = bacc.Bacc("TRN2")
    x = nc.dram_tensor("x_q", [H_PER_CORE, S, S], mybir.dt.int32, kind="ExternalInput")
    sx_d = nc.dram_tensor(
        "scale_x", [H_PER_CORE, S], mybir.dt.float32, kind="ExternalInput"
    )
    so_d = nc.dram_tensor(
        "scale_out", [H_PER_CORE, S], mybir.dt.float32, kind="ExternalInput"
    )
    y = nc.dram_tensor("out_q", [H_PER_CORE, S, S], mybir.dt.int8, kind="ExternalOutput")

    # Additive causal mask for the diagonal 128x128 block: 0 at j<=i, -2^20
    # above the diagonal (exp underflows to exactly 0 after dequant scaling).
    mask_np = np.where(
        np.tril(np.ones((P, P), dtype=bool)), 0, -(2**20)
    ).astype(np.int32)
    mask_d = nc.inline_tensor(mask_np, name="tri_mask")
    ident_d = nc.inline_tensor(np.eye(32, dtype=np.float32), name="ident32")

    with tile.TileContext(nc) as tc:
        with (
            tc.tile_pool(name="xq", bufs=5) as xq_pool,
            tc.tile_pool(name="ex", bufs=4) as ex_pool,
            tc.tile_pool(name="oq", bufs=4) as oq_pool,
            tc.tile_pool(name="sc", bufs=2) as sc_pool,
            tc.tile_pool(name="col", bufs=16) as col_pool,
            tc.tile_pool(name="one", bufs=1) as one_pool,
        ):
            order = _ORDER
            PREFETCH = _PREFETCH

            issued = {}

            def issue_load(r):
                W = P * (r + 1)
                rows = slice(r * P, (r + 1) * P)
                xt = xq_pool.tile([P, H_PER_CORE, S], mybir.dt.int32, tag="xq")
                nc.sync.dma_start(
                    out=xt[:, :, :W],
                    in_=x[:, rows, :W].rearrange("h p w -> p h w"),
                )
                issued[r] = xt

            # Emit the first loads BEFORE the setup DMAs: the scheduler's
            # priority follows program order, so the big transfers start
            # streaming while setup trickles in behind them.
            for r in order[:PREFETCH]:
                issue_load(r)

            mask_t = one_pool.tile([P, P], mybir.dt.int32)
            nc.sync.dma_start(out=mask_t, in_=mask_d[:, :])
            # Scale tables: contiguous [32,128] loads (fast), then a PE
            # identity-transpose into partition-major [128,32] — column
            # 16h + r holds head h, row-tile r.  (A direct strided DMA into
            # [128, NT] layout is descriptor-bound: ~900ns each and they gate
            # the whole pipeline start.)
            idt = one_pool.tile([32, 32], mybir.dt.float32)
            nc.sync.dma_start(out=idt, in_=ident_d[:, :])
            sxs = one_pool.tile([32, P], mybir.dt.float32)
            sos = one_pool.tile([32, P], mybir.dt.float32)
            nc.sync.dma_start(out=sxs, in_=sx_d.rearrange("h (q p) -> (h q) p", p=P))
            nc.sync.dma_start(out=sos, in_=so_d.rearrange("h (q p) -> (h q) p", p=P))
            with tc.tile_pool(name="ps", bufs=1, space="PSUM") as ps_pool:
                psx = ps_pool.tile([P, 32], mybir.dt.float32)
                pso = ps_pool.tile([P, 32], mybir.dt.float32)
                nc.tensor.transpose(psx, sxs, idt)
                nc.tensor.transpose(pso, sos, idt)
                sxt_all = sc_pool.tile([P, 2 * NT], mybir.dt.float32, tag="sx")
                sot_all = sc_pool.tile([P, 2 * NT], mybir.dt.float32, tag="so")
                nc.vector.tensor_copy(sxt_all, psx)
                # Invert the requant scale once: the per-tile scale step
                # becomes a single fused tensor_scalar (x*(1/sum))*(1/so).
                nc.vector.reciprocal(sot_all, pso)
            sxts = [sxt_all[:, NT * h : NT * (h + 1)] for h in range(H_PER_CORE)]
            sots = [sot_all[:, NT * h : NT * (h + 1)] for h in range(H_PER_CORE)]

            for i, r in enumerate(order):
                W = P * (r + 1)
                rows = slice(r * P, (r + 1) * P)
                xt = issued.pop(r)
                if i + PREFETCH < len(order):
                    issue_load(order[i + PREFETCH])
                import concourse.bass as bass

                mask_b = bass.AP(
                    tensor=mask_t.tensor,
                    offset=mask_t.offset,
                    ap=[list(mask_t.ap[0]), [0, H_PER_CORE], list(mask_t.ap[1])],
                )
                nc.vector.tensor_add(
                    xt[:, :, W - P : W], xt[:, :, W - P : W], mask_b
                )
                et = ex_pool.tile([P, H_PER_CORE, S], mybir.dt.float32, tag="ex")
                ot = oq_pool.tile([P, H_PER_CORE, S], mybir.dt.int8, tag="oq")
                for h in range(H_PER_CORE):
                    ssum = col_pool.tile([P, 1], mybir.dt.float32, tag="col")
                    nc.scalar.activation(
                        out=et[:, h, :W],
                        in_=xt[:, h, :W],
                        func=mybir.ActivationFunctionType.Exp,
                        scale=sxts[h][:, r : r + 1],
                        accum_out=ssum,
                    )
                    fac = col_pool.tile([P, 1], mybir.dt.float32, tag="col")
                    nc.vector.reciprocal(fac, ssum)
                    nc.vector.tensor_scalar(
                        out=ot[:, h, :W],
                        in0=et[:, h, :W],
                        scalar1=fac,
                        scalar2=sots[h][:, r : r + 1],
                        op0=mybir.AluOpType.mult,
                        op1=mybir.AluOpType.mult,
                    )
                nc.sync.dma_start(
                    out=y[:, rows, :W].rearrange("h p w -> p h w"),
                    in_=ot[:, :, :W],
                )
    nc.compile()
    return nc


def kernel(x_q, scale_x, scale_out, _trace=False):
    from concourse.bass_utils import run_bass_kernel_spmd

    x_q = np.asarray(x_q)
    scale_x = np.asarray(scale_x)
    scale_out = np.asarray(scale_out)

    nc = _build()
    in_maps = []
    for c in range(N_CORES):
        h0 = c * H_PER_CORE
        in_maps.append(
            {
                "x_q": np.ascontiguousarray(x_q[0, h0 : h0 + H_PER_CORE]),
                "scale_x": np.ascontiguousarray(scale_x[h0 : h0 + H_PER_CORE]),
                "scale_out": np.ascontiguousarray(scale_out[h0 : h0 + H_PER_CORE]),
            }
        )
    res = run_bass_kernel_spmd(
        nc, in_maps, core_ids=list(range(N_CORES)), trace=_trace
    )
    kernel._last_results = res
    out_q = np.concatenate([r["out_q"] for r in res.results], axis=0)
    out_q = out_q.reshape(1, H, S, S).astype(np.int8)
    return out_q, scale_out[:, :S].astype(np.float32)
